# revision 1
# baseline (speedup 1.0000x reference)
"""Banded (sliding-window) GQA attention block on 8 trn2 cores.

Sharding: 8 cores = batch(4) x seq-halves(2). Each core computes 1024
queries for one batch element with a 127-position K/V halo on each side.
All layouts are transposed ([feature, seq]) so the tensor engine contracts
naturally; RoPE even/odd lanes are split into separate tensors (same
partitions) so the rotation is full-width DVE work.

Attention is computed in S.T layout via diagonal key-chunk blocks
[128 keys x 384 queries]; band masking is accumulated into PSUM with an
identity matmul; softmax denominators come from an appended ones-column
in V (so no max-subtraction: scores are small enough that raw exp fits
comfortably in f32).
"""

import sys

sys.path.insert(0, "/opt/trn_rl_repo")

import numpy as np

import concourse.bass as bass
from concourse import bacc
import concourse.mybir as mybir
import concourse.tile as tile
from concourse.bass_utils import run_bass_kernel_spmd
from concourse.masks import make_identity

B, S, D = 4, 2048, 1024
H, KVH, HD = 16, 2, 64
W, HWD = 255, 127
SL = S // 2              # local queries per core
U = SL + 2 * HWD + 2     # 1280 padded key columns (1278 + 2 round-up)
UQ = U + 256             # 1536: query tensors padded 128 each side
NCH = U // 128           # 10 key chunks
NEG = -1.0e30

f32 = mybir.dt.float32
f32r = mybir.dt.float32r
bf16 = mybir.dt.bfloat16


def build_nc():
    nc = bacc.Bacc("TRN2")
    dp = nc.declare_dram_parameter
    xT = dp("xT", [D, U], f32r, isOutput=False)
    wqe = dp("wqe", [D, 512], f32r, isOutput=False)
    wqo = dp("wqo", [D, 512], f32r, isOutput=False)
    wke = dp("wke", [D, 256], f32r, isOutput=False)
    wko = dp("wko", [D, 256], f32r, isOutput=False)
    wv = dp("wv", [D, 128], f32r, isOutput=False)
    wo = dp("wo", [D, D], f32r, isOutput=False)
    bqe = dp("bqe", [1, 512], f32r, isOutput=False)
    bqo = dp("bqo", [1, 512], f32r, isOutput=False)
    bke = dp("bke", [1, 256], f32r, isOutput=False)
    bko = dp("bko", [1, 256], f32r, isOutput=False)
    bvb = dp("bvb", [1, 128], f32r, isOutput=False)
    bob = dp("bob", [1, D], f32r, isOutput=False)
    cosq = dp("cosq", [128, U], f32, isOutput=False)
    sinq = dp("sinq", [128, U], f32, isOutput=False)
    cosk = dp("cosk", [128, U], f32, isOutput=False)
    sink = dp("sink", [128, U], f32, isOutput=False)
    maskT = dp("maskT", [128, 384], f32r, isOutput=False)
    out = dp("out", [SL, D], f32, isOutput=True)

    NB = [(0, 512), (512, 512), (1024, 256)]  # N-blocks over U

    with tile.TileContext(nc) as tc:
        with (
            nc.allow_low_precision(reason="f32r tiles are 4-byte; elementwise ops only"),
            tc.tile_pool(name="persist", bufs=1) as pe,
        ):
            # ---- persistent SBUF ----
            ident_f = pe.tile([128, 128], f32, tag="identf")
            make_identity(nc, ident_f)
            ident = pe.tile([128, 128], f32r, tag="ident")
            nc.vector.tensor_copy(ident[:], ident_f[:])
            ones_f = pe.tile([1, 512], f32, tag="onesf")
            nc.vector.memset(ones_f[:], 1.0)
            ones = pe.tile([1, 512], f32r, tag="ones")
            nc.vector.tensor_copy(ones[:], ones_f[:])
            mask_t = pe.tile([128, 384], f32r, tag="mask")
            nc.gpsimd.dma_start(mask_t[:], maskT[:])
            qte = [pe.tile([128, UQ], f32r, tag=f"qte{g}", name=f"qte{g}") for g in range(4)]
            qto = [pe.tile([128, UQ], f32r, tag=f"qto{g}", name=f"qto{g}") for g in range(4)]
            ktr_e = [pe.tile([128, U], f32r, tag=f"kte{b2}", name=f"kte{b2}") for b2 in range(2)]
            ktr_o = [pe.tile([128, U], f32r, tag=f"kto{b2}", name=f"kto{b2}") for b2 in range(2)]
            vaug = [pe.tile([128, 65 * NCH], bf16, tag=f"vaug{k}", name=f"vaug{k}") for k in range(2)]
            bo_sb = pe.tile([1, D], f32r, tag="bo")
            nc.gpsimd.dma_start(bo_sb[:], bob[:])

            for k in range(2):
                nc.vector.memset(vaug[k][:], 1.0)
            zf = pe.tile([128, 128], f32, tag="zf")
            nc.vector.memset(zf[:], 0.0)
            for g in range(4):
                nc.vector.tensor_copy(qte[g][:, 0:128], zf[:])
                nc.vector.tensor_copy(qte[g][:, UQ - 128 : UQ], zf[:])
                nc.vector.tensor_copy(qto[g][:, 0:128], zf[:])
                nc.vector.tensor_copy(qto[g][:, UQ - 128 : UQ], zf[:])

            # ================= phase A: projections + rope =================
            with (
                tc.tile_pool(name="proj_in", bufs=1) as pin,
                tc.tile_pool(name="tmp", bufs=1) as ptmp,
                tc.tile_pool(name="qps", bufs=2, space="PSUM") as qps,
                tc.tile_pool(name="vps", bufs=2, space="PSUM") as vps,
            ):
                xts = [pin.tile([128, U], f32r, tag=f"x{i}", name=f"x{i}") for i in range(8)]
                for i in range(8):
                    nc.gpsimd.dma_start(xts[i][:], xT[128 * i : 128 * i + 128, :])
                wke_s = [pin.tile([128, 256], f32r, tag=f"wke{i}", name=f"wke{i}") for i in range(8)]
                wko_s = [pin.tile([128, 256], f32r, tag=f"wko{i}", name=f"wko{i}") for i in range(8)]
                wv_s = [pin.tile([128, 128], f32r, tag=f"wv{i}", name=f"wv{i}") for i in range(8)]
                for i in range(8):
                    sl = slice(128 * i, 128 * i + 128)
                    nc.gpsimd.dma_start(wke_s[i][:], wke[sl, :])
                    nc.gpsimd.dma_start(wko_s[i][:], wko[sl, :])
                    nc.gpsimd.dma_start(wv_s[i][:], wv[sl, :])
                cq = pin.tile([128, U], f32, tag="cq")
                sq = pin.tile([128, U], f32, tag="sq")
                ck = pin.tile([128, U], f32, tag="ck")
                sk = pin.tile([128, U], f32, tag="sk")
                nc.gpsimd.dma_start(cq[:], cosq[:])
                nc.gpsimd.dma_start(sq[:], sinq[:])
                nc.gpsimd.dma_start(ck[:], cosk[:])
                nc.gpsimd.dma_start(sk[:], sink[:])
                be_s = pin.tile([1, 512], f32r, tag="bqe")
                bo_s2 = pin.tile([1, 512], f32r, tag="bqo")
                bke_s = pin.tile([1, 256], f32r, tag="bke")
                bko_s = pin.tile([1, 256], f32r, tag="bko")
                bv_s = pin.tile([1, 128], f32r, tag="bv")
                nc.gpsimd.dma_start(be_s[:], bqe[:])
                nc.gpsimd.dma_start(bo_s2[:], bqo[:])
                nc.gpsimd.dma_start(bke_s[:], bke[:])
                nc.gpsimd.dma_start(bko_s[:], bko[:])
                nc.gpsimd.dma_start(bv_s[:], bvb[:])

                def proj(ps, wtiles, wsl, btile, bsl, nrows):
                    # ps[(0:nrows), 0:U] = (w slice).T @ xT + bias
                    for n0, nw in NB:
                        for kc in range(8):
                            nc.tensor.matmul(
                                ps[0:nrows, n0 : n0 + nw],
                                wtiles[kc][:, wsl],
                                xts[kc][:, n0 : n0 + nw],
                                start=(kc == 0),
                                stop=False,
                            )
                        nc.tensor.matmul(
                            ps[0:nrows, n0 : n0 + nw],
                            btile[0:1, bsl],
                            ones[0:1, 0:nw],
                            start=False,
                            stop=True,
                        )

                def rope(ps_e, ps_o, dst_e, dst_o, c, s, nrows, width, dcol):
                    t1 = ptmp.tile([128, U], f32, tag="t1")
                    t2 = ptmp.tile([128, U], f32, tag="t2")
                    r = slice(0, nrows)
                    w = slice(0, width)
                    cc = c[r, 0:width]
                    ss = s[r, 0:width]
                    mult, add, sub = (
                        mybir.AluOpType.mult,
                        mybir.AluOpType.add,
                        mybir.AluOpType.subtract,
                    )
                    nc.vector.tensor_tensor(t1[r, w], ps_e[r, w], cc, mult)
                    nc.vector.tensor_tensor(t2[r, w], ps_o[r, w], ss, mult)
                    nc.vector.tensor_tensor(
                        dst_e[r, dcol : dcol + width], t1[r, w], t2[r, w], sub
                    )
                    t3 = ptmp.tile([128, U], f32, tag="t1")
                    t4 = ptmp.tile([128, U], f32, tag="t2")
                    nc.vector.tensor_tensor(t3[r, w], ps_e[r, w], ss, mult)
                    nc.vector.tensor_tensor(t4[r, w], ps_o[r, w], cc, mult)
                    nc.vector.tensor_tensor(
                        dst_o[r, dcol : dcol + width], t3[r, w], t4[r, w], add
                    )

                for b2 in range(2):
                    ps_e = qps.tile([128, U], f32, tag="qp")
                    ps_o = qps.tile([128, U], f32, tag="qp")
                    proj(ps_e, wke_s, slice(128 * b2, 128 * b2 + 128), bke_s,
                         slice(128 * b2, 128 * b2 + 128), 128)
                    proj(ps_o, wko_s, slice(128 * b2, 128 * b2 + 128), bko_s,
                         slice(128 * b2, 128 * b2 + 128), 128)
                    rope(ps_e, ps_o, ktr_e[b2], ktr_o[b2], ck, sk, 128, U, 0)

                # V projection (f32 for accuracy), ones column kept at 1.0
                for sti in range(NCH):
                    vp = vps.tile([128, 128], f32, tag="vp")
                    ssl = slice(128 * sti, 128 * sti + 128)
                    for kc in range(8):
                        nc.tensor.matmul(
                            vp[:], xts[kc][:, ssl], wv_s[kc][:], start=(kc == 0),
                            stop=False,
                        )
                    nc.tensor.matmul(
                        vp[:], ones[0:1, 0:128], bv_s[:], start=False, stop=True
                    )
                    for k in range(2):
                        nc.scalar.copy(
                            vaug[k][:, 65 * sti : 65 * sti + 64],
                            vp[:, 64 * k : 64 * k + 64],
                        )

                for gh in range(2):
                    wqe_s = [pin.tile([128, 256], f32r, tag=f"wqe{i}", name=f"wqeh{i}") for i in range(8)]
                    wqo_s = [pin.tile([128, 256], f32r, tag=f"wqo{i}", name=f"wqoh{i}") for i in range(8)]
                    for i in range(8):
                        sl = slice(128 * i, 128 * i + 128)
                        nc.gpsimd.dma_start(wqe_s[i][:], wqe[sl, 256 * gh : 256 * gh + 256])
                        nc.gpsimd.dma_start(wqo_s[i][:], wqo[sl, 256 * gh : 256 * gh + 256])
                    for g in (2 * gh, 2 * gh + 1):
                        ps_e = qps.tile([128, U], f32, tag="qp")
                        ps_o = qps.tile([128, U], f32, tag="qp")
                        proj(ps_e, wqe_s, slice(128 * (g % 2), 128 * (g % 2) + 128), be_s,
                             slice(128 * g, 128 * g + 128), 128)
                        proj(ps_o, wqo_s, slice(128 * (g % 2), 128 * (g % 2) + 128), bo_s2,
                             slice(128 * g, 128 * g + 128), 128)
                        rope(ps_e, ps_o, qte[g], qto[g], cq, sq, 128, U, 128)

            # ============ phase B: scores -> exp -> PV -> normalize ============
            with tc.tile_pool(name="pattn", bufs=1) as pattn:
              attn = [pattn.tile([128, U], f32r, tag=f"attn{t}", name=f"attn{t}") for t in range(8)]
              with (
                tc.tile_pool(name="spool", bufs=2, space="PSUM") as spool,
                tc.tile_pool(name="ppool", bufs=3) as ppool,
                tc.tile_pool(name="npool", bufs=3) as npool,
                tc.tile_pool(name="ppv", bufs=3, space="PSUM") as ppv,
                tc.tile_pool(name="prb", bufs=1, space="PSUM") as prb,
                tc.tile_pool(name="pqx", bufs=1) as pqx,
              ):
                for h in range(H):
                    kv = h // 8
                    gq = h // 4
                    if h % 4 == 3:
                        qxe = pqx.tile([32, UQ], f32r, tag="qxe", name="qxe")
                        qxo = pqx.tile([32, UQ], f32r, tag="qxo", name="qxo")
                        nc.vector.tensor_copy(qxe[:], qte[gq][96:128, :])
                        nc.vector.tensor_copy(qxo[:], qto[gq][96:128, :])
                        qe_t, qo_t, rq = qxe, qxo, slice(0, 32)
                    else:
                        qe_t, qo_t, rq = qte[gq], qto[gq], slice(32 * (h % 4), 32 * (h % 4) + 32)
                    pv_ps = {}
                    for m in range(3):
                        pv_ps[m] = ppv.tile([128, 512], f32, tag="pv", name=f"pv{m}")

                    pts = {}
                    for p in range(NCH // 2):
                        sp = spool.tile([128, 1024], f32, tag="sc")
                        pt = ppool.tile([128, 768], bf16, tag="pt")
                        pts[p] = pt
                        for half in range(2):
                            c = 2 * p + half
                            c0 = 128 * c
                            col = 512 * half
                            nc.tensor.matmul(
                                sp[:, col : col + 384],
                                ktr_e[kv][rq, c0 : c0 + 128],
                                qe_t[rq, c0 : c0 + 384],
                                start=True, stop=False,
                            )
                            nc.tensor.matmul(
                                sp[:, col : col + 384],
                                ktr_o[kv][rq, c0 : c0 + 128],
                                qo_t[rq, c0 : c0 + 384],
                                start=False, stop=False,
                            )
                            nc.tensor.matmul(
                                sp[:, col : col + 384],
                                ident[:],
                                mask_t[:],
                                start=False, stop=True,
                            )
                        sview = sp[:].rearrange("p (b x) -> p b x", b=2)[:, :, 0:384]
                        pview = pt[:].rearrange("p (b x) -> p b x", b=2)
                        nc.scalar.activation(
                            pview, sview, mybir.ActivationFunctionType.Exp
                        )
                        lo = max(0, 2 * p - 1)
                        hi = 2 * p if p < NCH // 2 - 1 else NCH - 1
                        for j in range(lo, hi + 1):
                            for c in (j - 1, j, j + 1):
                                if c < 0 or c >= NCH:
                                    continue
                                pt_c = pts[c // 2]
                                base = 384 * (c % 2) + 128 * (j - c + 1)
                                nc.tensor.matmul(
                                    pv_ps[j // 4][0:65, 128 * (j % 4) : 128 * (j % 4) + 128],
                                    vaug[kv][:, 65 * c : 65 * c + 65],
                                    pt_c[:, base : base + 128],
                                    start=(c == max(0, j - 1)),
                                    stop=(c == min(NCH - 1, j + 1)),
                                )

                    # normalize: attn[t] rows = out.T rows for this head
                    t = h // 2
                    r0 = 64 * (h % 2)
                    for m in range(3):
                        wdt = 512 if m < 2 else 256
                        rd = npool.tile([1, 512], f32r, tag="rd")
                        nc.vector.reciprocal(rd[0:1, 0:wdt], pv_ps[m][64:65, 0:wdt])
                        rb_ps = prb.tile([128, 512], f32, tag="rb")
                        nc.tensor.matmul(
                            rb_ps[0:64, 0:wdt],
                            ones[0:1, 0:64],
                            rd[0:1, 0:wdt],
                            start=True, stop=True,
                        )
                        rb_sb = npool.tile([64, 512], f32, tag="rbs")
                        nc.scalar.copy(rb_sb[0:64, 0:wdt], rb_ps[0:64, 0:wdt])
                        nc.vector.tensor_tensor(
                            attn[t][r0 : r0 + 64, 512 * m : 512 * m + wdt],
                            pv_ps[m][0:64, 0:wdt],
                            rb_sb[0:64, 0:wdt],
                            mybir.AluOpType.mult,
                        )

              # ================= phase C: output projection =================
              with (
                  tc.tile_pool(name="wop", bufs=1) as pwo,
                  tc.tile_pool(name="oout", bufs=3) as pou,
                  tc.tile_pool(name="ops", bufs=2, space="PSUM") as ops,
              ):
                  wo_s = [pwo.tile([128, D], f32r, tag=f"wo{i}", name=f"wo{i}") for i in range(8)]
                  for i in range(8):
                      nc.gpsimd.dma_start(wo_s[i][:], wo[128 * i : 128 * i + 128, :])
                  for tq in range(8):
                      q0 = 127 + 128 * tq
                      for nb in range(2):
                          op = ops.tile([128, 512], f32, tag="op")
                          for kc in range(8):
                              nc.tensor.matmul(
                                  op[:],
                                  attn[kc][:, q0 : q0 + 128],
                                  wo_s[kc][:, 512 * nb : 512 * nb + 512],
                                  start=(kc == 0), stop=False,
                              )
                          nc.tensor.matmul(
                              op[:],
                              ones[0:1, 0:128],
                              bo_sb[0:1, 512 * nb : 512 * nb + 512],
                              start=False, stop=True,
                          )
                          ot = pou.tile([128, 512], f32, tag="ot")
                          nc.scalar.copy(ot[:], op[:])
                          nc.sync.dma_start(
                              out[128 * tq : 128 * tq + 128, 512 * nb : 512 * nb + 512],
                              ot[:],
                          )
    nc.finalize()
    return nc


_PERM_QE = np.array(
    [(4 * g + a) * 64 + 2 * i for g in range(4) for a in range(4) for i in range(32)]
)
_PK = [np.array([kv * 64 + 2 * i for i in range(32)]) for kv in range(2)]


def make_inputs(x, freqs_cis, w_q, b_q, w_k, b_k, w_v, b_v, w_o, b_o):
    cos = np.asarray(freqs_cis[..., 0], dtype=np.float32)  # (S, 32)
    sin = np.asarray(freqs_cis[..., 1], dtype=np.float32)
    x = np.asarray(x, dtype=np.float32)
    maskT = np.full((128, 384), NEG, dtype=np.float32)
    for k in range(128):
        maskT[k, k + 1 : k + 256] = 0.0
    common = dict(
        wqe=np.ascontiguousarray(w_q[:, _PERM_QE]),
        wqo=np.ascontiguousarray(w_q[:, _PERM_QE + 1]),
        wke=np.concatenate([np.tile(w_k[:, _PK[kv]], (1, 4)) for kv in range(2)], 1),
        wko=np.concatenate([np.tile(w_k[:, _PK[kv] + 1], (1, 4)) for kv in range(2)], 1),
        wv=np.ascontiguousarray(w_v),
        wo=np.ascontiguousarray(w_o),
        bqe=b_q[_PERM_QE][None, :].astype(np.float32),
        bqo=b_q[_PERM_QE + 1][None, :].astype(np.float32),
        bke=np.concatenate([np.tile(b_k[_PK[kv]], 4) for kv in range(2)])[None, :].astype(np.float32),
        bko=np.concatenate([np.tile(b_k[_PK[kv] + 1], 4) for kv in range(2)])[None, :].astype(np.float32),
        bvb=np.asarray(b_v, dtype=np.float32)[None, :],
        bob=np.asarray(b_o, dtype=np.float32)[None, :],
        maskT=maskT,
    )
    maps = []
    for c in range(8):
        b, hf = c // 2, c % 2
        s0 = SL * hf
        pos = s0 - HWD + np.arange(U)
        valid = (pos >= 0) & (pos < S)
        pc = np.clip(pos, 0, S - 1)
        xTc = np.where(valid[None, :], x[b][pc].T, 0.0).astype(np.float32)
        ckc = np.tile(cos[pc].T, (4, 1)).astype(np.float32)
        skc = np.tile(sin[pc].T, (4, 1)).astype(np.float32)
        cq = np.tile(cos[pc].T, (4, 1)).astype(np.float32)
        sq = np.tile(sin[pc].T, (4, 1)).astype(np.float32)
        m = dict(common)
        m.update(xT=xTc, cosq=cq, sinq=sq, cosk=ckc, sink=skc)
        maps.append(m)
    return maps


_NC_CACHE = {}


def kernel(x, freqs_cis, w_q, b_q, w_k, b_k, w_v, b_v, w_o, b_o):
    if "nc" not in _NC_CACHE:
        _NC_CACHE["nc"] = build_nc()
    nc = _NC_CACHE["nc"]
    maps = make_inputs(
        np.asarray(x), np.asarray(freqs_cis), np.asarray(w_q), np.asarray(b_q),
        np.asarray(w_k), np.asarray(b_k), np.asarray(w_v), np.asarray(b_v),
        np.asarray(w_o), np.asarray(b_o),
    )
    res = run_bass_kernel_spmd(nc, maps, list(range(8))).results
    full = np.empty((B, S, D), np.float32)
    for c in range(8):
        b, hf = c // 2, c % 2
        full[b, SL * hf : SL * (hf + 1), :] = res[c]["out"]
    return full



# revision 9
# speedup vs baseline: 1.8990x; 1.8990x over previous
"""Banded (sliding-window) GQA attention block on 8 trn2 cores.

Sharding: 8 cores = batch(4) x seq-halves(2). Each core computes 1024
queries for one batch element with a 128-position K/V halo on each side
(window half = 127, padded to 128 so everything is 128-aligned).

Layouts are transposed ([feature, seq]) so the tensor engine contracts
naturally. RoPE even/odd lanes are packed per head into 64 contiguous
partitions ([e0..e31, o0..o31]) so each score block is a single K=64
matmul; the rotation is computed as P1*C + P2*S' where P2 is a 32-row
pair-swap of the projection PSUM obtained with one permutation matmul.

Band masking multiplies the bf16 probabilities with a 0/1 band tile on
the vector engine (2x 16-bit mode) instead of a -inf matmul. Softmax
denominators come from an appended ones-column in V; no max-subtraction
(scores are small enough that raw exp fits in f32).
"""

import sys

sys.path.insert(0, "/opt/trn_rl_repo")

import numpy as np

import concourse.bass as bass
from concourse import bacc
import concourse.mybir as mybir
import concourse.tile as tile
from concourse.bass_utils import run_bass_kernel_spmd
from concourse.masks import make_identity

B, S, D = 4, 2048, 1024
H, KVH, HD = 16, 2, 64
W, HWD = 255, 127
SL = S // 2              # local queries per core
PAD = 128                # left/right key padding (>= half window, 128-aligned)
U = SL + 2 * PAD         # 1280 padded key columns
UQ = U + 256             # 1536: query tensors padded 128 each side
NCH = U // 128           # 10 key chunks

f32 = mybir.dt.float32
f32r = mybir.dt.float32r
bf16 = mybir.dt.bfloat16

Exp = mybir.ActivationFunctionType.Exp
Ident = mybir.ActivationFunctionType.Identity
MULT = mybir.AluOpType.mult
ADD = mybir.AluOpType.add


def build_nc():
    nc = bacc.Bacc("TRN2")
    dp = nc.declare_dram_parameter
    xT = dp("xT", [D, U], f32r, isOutput=False)
    wqm = dp("wqm", [D, 1024], f32r, isOutput=False)
    wkm = dp("wkm", [D, 128], f32r, isOutput=False)
    wvm = dp("wvm", [D, 128], f32r, isOutput=False)
    wom = dp("wom", [D, D], f32r, isOutput=False)
    cqt = dp("cqt", [128, SL], f32, isOutput=False)
    sqt = dp("sqt", [128, SL], f32, isOutput=False)
    ckt = dp("ckt", [128, U], f32, isOutput=False)
    skt = dp("skt", [128, U], f32, isOutput=False)
    permm = dp("permm", [128, 128], f32r, isOutput=False)
    bandm = dp("bandm", [128, 768], f32, isOutput=False)
    bqc = dp("bqc", [128, 8], f32, isOutput=False)
    bkc = dp("bkc", [128, 1], f32, isOutput=False)
    bvc = dp("bvc", [128, 1], f32, isOutput=False)
    boc = dp("boc", [1, D], f32r, isOutput=False)
    out = dp("out", [SL, D], f32, isOutput=True)

    NB = [(0, 512), (512, 512), (1024, 256)]  # N-blocks over U
    NBQ = [(0, 512), (512, 512)]              # N-blocks over SL

    with tile.TileContext(nc) as tc:
        with (
            nc.allow_low_precision(reason="f32r tiles are 4-byte; elementwise ops only"),
            tc.tile_pool(name="persist", bufs=1) as pe,
        ):
            # ---- persistent SBUF ----
            ident_f = pe.tile([128, 128], f32, tag="identf")
            make_identity(nc, ident_f)
            ident = pe.tile([128, 128], f32r, tag="ident")
            nc.vector.tensor_copy(ident[:], ident_f[:])
            ones_f = pe.tile([1, 512], f32, tag="onesf")
            nc.vector.memset(ones_f[:], 1.0)
            ones = pe.tile([1, 512], f32r, tag="ones")
            nc.vector.tensor_copy(ones[:], ones_f[:])

            # small parameter tiles (issue DMAs early; tiny transfers)
            bvc_sb = pe.tile([128, 1], f32, tag="bvc")
            bkc_sb = pe.tile([128, 1], f32, tag="bkc")
            bqc_sb = pe.tile([128, 8], f32, tag="bqc")
            bo_sb = pe.tile([1, D], f32r, tag="bo")
            nc.gpsimd.dma_start(bvc_sb[:], bvc[:])
            nc.gpsimd.dma_start(bkc_sb[:], bkc[:])
            nc.gpsimd.dma_start(bqc_sb[:], bqc[:])
            nc.gpsimd.dma_start(bo_sb[:], boc[:])

            qm = [pe.tile([128, UQ], f32r, tag=f"qm{t}", name=f"qm{t}") for t in range(8)]
            ktr = pe.tile([128, U], f32r, tag="ktr")
            vaug = [pe.tile([128, 65 * NCH], bf16, tag=f"vaug{k}", name=f"vaug{k}") for k in range(2)]
            for k in range(2):
                nc.vector.memset(vaug[k][:], 1.0)
            # zero the query padding wings (only cols 256:1280 get written)
            for t in range(8):
                nc.vector.memset(qm[t][:, 0:256], 0.0)
                nc.vector.memset(qm[t][:, UQ - 256 : UQ], 0.0)

            perm_sb = pe.tile([128, 128], f32r, tag="perm")
            band_f = pe.tile([128, 768], f32, tag="bandf")
            band = pe.tile([128, 768], bf16, tag="band")

            # ================= phase A: projections + rope =================
            with (
                tc.tile_pool(name="proj_in", bufs=1) as pin,
                tc.tile_pool(name="ptmp", bufs=1) as ptmp,
            ):
                wv_sb = pin.tile([128, 1024], f32r, tag="wv")
                xts = [pin.tile([128, U], f32r, tag=f"x{i}", name=f"x{i}") for i in range(8)]
                wkm_sb = pin.tile([128, 1024], f32r, tag="wkm")
                wqm_sb = pin.tile([128, 8192], f32r, tag="wqm")
                cq_sb = pin.tile([128, SL], f32, tag="cq")
                sq_sb = pin.tile([128, SL], f32, tag="sq")
                ck_sb = pin.tile([128, U], f32, tag="ck")
                sk_sb = pin.tile([128, U], f32, tag="sk")

                # DMA issue order == just-in-time consumption order
                v_kc = lambda kc: slice(128 * kc, 128 * kc + 128)
                nc.gpsimd.dma_start(
                    wv_sb[:].rearrange("p (kc f) -> p kc f", kc=8),
                    wvm[:].rearrange("(kc p) f -> p kc f", kc=8),
                )
                for i in range(4):
                    nc.gpsimd.dma_start(xts[i][:], xT[v_kc(i), :])
                nc.gpsimd.dma_start(
                    wkm_sb[:].rearrange("p (kc f) -> p kc f", kc=8),
                    wkm[:].rearrange("(kc p) f -> p kc f", kc=8),
                )
                for i in range(4, 8):
                    nc.gpsimd.dma_start(xts[i][:], xT[v_kc(i), :])
                nc.gpsimd.dma_start(ck_sb[:], ckt[:])
                nc.gpsimd.dma_start(sk_sb[:], skt[:])
                nc.gpsimd.dma_start(perm_sb[:], permm[:])
                for i in range(8):
                    nc.gpsimd.dma_start(wqm_sb[:, 1024 * i : 1024 * i + 1024], wqm[v_kc(i), :])
                    if i == 1:
                        nc.gpsimd.dma_start(cq_sb[:], cqt[:])
                        nc.gpsimd.dma_start(sq_sb[:], sqt[:])
                nc.gpsimd.dma_start(band_f[:], bandm[:])
                nc.vector.tensor_copy(band[:], band_f[:])

                # ---- V projection, transposed: Vt[vd, seq] then per-chunk T ----
                with (
                    tc.tile_pool(name="vps", bufs=1, space="PSUM") as vps,
                    tc.tile_pool(name="tps", bufs=2, space="PSUM") as tps,
                ):
                    vt_ps = vps.tile([128, U], f32, tag="vt")
                    for n0, nw in NB:
                        for kc in range(8):
                            nc.tensor.matmul(
                                vt_ps[:, n0 : n0 + nw],
                                wv_sb[:, v_kc(kc)],
                                xts[kc][:, n0 : n0 + nw],
                                start=(kc == 0),
                                stop=(kc == 7),
                            )
                    vt_sb = pin.tile([128, U], f32r, tag="vts")
                    nc.scalar.activation(vt_sb[:], vt_ps[:], Ident, bias=bvc_sb[:, 0:1])
                    for g in range(3):
                        tp = tps.tile([128, 512], f32r, tag="tp")
                        for s in range(8 if g < 2 else 4):
                            st, kvh = (8 * g + s) // 2, (8 * g + s) % 2
                            # identity block at the same partition base as the input
                            isl = slice(64 * kvh, 64 * kvh + 64)
                            nc.tensor.matmul(
                                tp[:, 64 * s : 64 * s + 64],
                                vt_sb[isl, 128 * st : 128 * st + 128],
                                ident[isl, isl],
                                is_transpose=True,
                            )
                            nc.scalar.copy(
                                vaug[kvh][:, 65 * st : 65 * st + 64],
                                tp[:, 64 * s : 64 * s + 64],
                            )

                # ---- K projection (both kv heads, merged e/o lanes) ----
                with tc.tile_pool(name="kps", bufs=1, space="PSUM") as kps:
                    p1k = kps.tile([128, U], f32, tag="p1k")
                    for n0, nw in NB:
                        for kc in range(8):
                            nc.tensor.matmul(
                                p1k[:, n0 : n0 + nw],
                                wkm_sb[:, v_kc(kc)],
                                xts[kc][:, n0 : n0 + nw],
                                start=(kc == 0),
                                stop=(kc == 7),
                            )
                    p1k_sb = pin.tile([128, U], f32r, tag="p1ks")
                    nc.scalar.activation(p1k_sb[:], p1k[:], Ident, bias=bkc_sb[:, 0:1])
                    p2k = kps.tile([128, U], f32, tag="p2k")
                    for n0, nw in NB:
                        nc.tensor.matmul(
                            p2k[:, n0 : n0 + nw], perm_sb[:], p1k_sb[:, n0 : n0 + nw],
                            start=True, stop=True,
                        )
                    tk1 = ptmp.tile([128, U], f32, tag="tk1")
                    tk2 = ptmp.tile([128, U], f32, tag="tk2")
                    nc.vector.tensor_tensor(tk1[:], p1k_sb[:], ck_sb[:], MULT)
                    nc.vector.tensor_tensor(tk2[:], p2k[:], sk_sb[:], MULT)
                    nc.vector.tensor_tensor(ktr[:], tk1[:], tk2[:], ADD)

                # ---- Q projection: 8 merged tiles (2 heads each) ----
                with tc.tile_pool(name="qps", bufs=2, space="PSUM") as qps:
                    for t in range(8):
                        p1 = qps.tile([128, SL], f32, tag="p1")
                        for n0, nw in NBQ:
                            for kc in range(8):
                                nc.tensor.matmul(
                                    p1[:, n0 : n0 + nw],
                                    wqm_sb[:, 1024 * kc + 128 * t : 1024 * kc + 128 * t + 128],
                                    xts[kc][:, 128 + n0 : 128 + n0 + nw],
                                    start=(kc == 0),
                                    stop=(kc == 7),
                                )
                        p1_sb = ptmp.tile([128, SL], f32r, tag="p1s")
                        nc.scalar.activation(p1_sb[:], p1[:], Ident, bias=bqc_sb[:, t : t + 1])
                        p2 = qps.tile([128, SL], f32, tag="p2")
                        for n0, nw in NBQ:
                            nc.tensor.matmul(
                                p2[:, n0 : n0 + nw], perm_sb[:], p1_sb[:, n0 : n0 + nw],
                                start=True, stop=True,
                            )
                        t1 = ptmp.tile([128, SL], f32, tag="t1")
                        t2 = ptmp.tile([128, SL], f32, tag="t2")
                        nc.vector.tensor_tensor(t1[:], p1_sb[:], cq_sb[:], MULT)
                        nc.vector.tensor_tensor(t2[:], p2[:], sq_sb[:], MULT)
                        nc.vector.tensor_tensor(qm[t][:, 256 : 256 + SL], t1[:], t2[:], ADD)

            # ============ phase B: scores -> exp -> mask -> PV -> normalize ============
            with tc.tile_pool(name="pattn", bufs=1) as pattn:
              attn = [pattn.tile([128, SL], f32r, tag=f"attn{t}", name=f"attn{t}") for t in range(8)]
              wo_sb = pattn.tile([128, 8192], f32r, tag="wo")
              for i in range(8):
                  nc.gpsimd.dma_start(
                      wo_sb[:, 1024 * i : 1024 * i + 1024], wom[128 * i : 128 * i + 128, :]
                  )
              biasrep = pattn.tile([128, 1024], f32r, tag="brep")
              with (
                tc.tile_pool(name="spool", bufs=2, space="PSUM") as spool,
                tc.tile_pool(name="ppool", bufs=6) as ppool,
                tc.tile_pool(name="npool", bufs=4) as npool,
                tc.tile_pool(name="ppv", bufs=3, space="PSUM") as ppv,
                tc.tile_pool(name="prb", bufs=1, space="PSUM") as prb,
              ):
                # bias-replica for phase C (built once on PE + act)
                br_ps = prb.tile([128, 512], f32, tag="rb")
                for nb2 in range(2):
                    nc.tensor.matmul(
                        br_ps[:], ones[0:1, 0:128], bo_sb[0:1, 512 * nb2 : 512 * nb2 + 512],
                        start=True, stop=True,
                    )
                    nc.scalar.copy(biasrep[:, 512 * nb2 : 512 * nb2 + 512], br_ps[:])

                deferred = []
                for h in range(H):
                    # qm[t] hosts heads (t, t+8): a head's 64 query lanes sit at
                    # partition base 64*kv, matching its kv rows in ktr.
                    t, kv = h % 8, h // 8
                    r0 = 64 * kv
                    at, ar0 = h // 2, 64 * (h % 2)  # attn feature rows for head h
                    pts = {}
                    pv_ps = [ppv.tile([128, 512], f32, tag="pv", name=f"pv{h}_{m}") for m in range(2)]
                    rb = prb.tile([128, 512], f32, tag="rb")

                    def sc(p):
                        sp = spool.tile([128, 1024], f32, tag="sc")
                        for half in range(2):
                            c = 2 * p + half
                            nc.tensor.matmul(
                                sp[:, 512 * half : 512 * half + 384],
                                ktr[64 * kv : 64 * kv + 64, 128 * c : 128 * c + 128],
                                qm[t][r0 : r0 + 64, 128 * c : 128 * c + 384],
                                start=True, stop=True,
                            )
                        pt = ppool.tile([128, 768], bf16, tag="pt")
                        nc.scalar.activation(
                            pt[:].rearrange("p (b x) -> p b x", b=2),
                            sp[:].rearrange("p (b x) -> p b x", b=2)[:, :, 0:384],
                            Exp,
                        )
                        nc.vector.tensor_tensor(pt[:], pt[:], band[:], MULT)
                        pts[p] = pt

                    def pv(j):
                        m, sl8 = (j - 1) // 4, 128 * ((j - 1) % 4)
                        for c in (j - 1, j, j + 1):
                            nc.tensor.matmul(
                                pv_ps[m][0:65, sl8 : sl8 + 128],
                                vaug[kv][:, 65 * c : 65 * c + 65],
                                pts[c // 2][:, 384 * (c % 2) + 128 * (j - c + 1) :
                                             384 * (c % 2) + 128 * (j - c + 1) + 128],
                                start=(c == j - 1),
                                stop=(c == j + 1),
                            )

                    def recip(m):
                        rd = npool.tile([1, 512], f32r, tag="rd")
                        nc.vector.reciprocal(rd[0:1, :], pv_ps[m][64:65, 0:512])
                        return rd

                    def rb_mult(m, rd, pv_t, at, ar0):
                        nc.tensor.matmul(
                            rb[64 * m : 64 * m + 64, :], ones[0:1, 0:64], rd[0:1, :],
                            start=True, stop=True,
                        )
                        nc.vector.tensor_tensor(
                            attn[at][ar0 : ar0 + 64, 512 * m : 512 * m + 512],
                            pv_t[0:64, 0:512],
                            rb[64 * m : 64 * m + 64, :],
                            MULT,
                        )

                    sc(0)
                    sc(1)
                    for fn in deferred:
                        fn()
                    deferred = []
                    pv(1); pv(2)
                    sc(2)
                    pv(3); pv(4)
                    rd0 = recip(0)
                    sc(3)
                    pv(5); pv(6)
                    rb_mult(0, rd0, pv_ps[0], at, ar0)
                    sc(4)
                    pv(7); pv(8)
                    rd1 = recip(1)
                    deferred.append(
                        lambda m=1, rd=rd1, pv_t=pv_ps[1], a=at, a0=ar0: rb_mult(m, rd, pv_t, a, a0)
                    )
                for fn in deferred:
                    fn()

              # ================= phase C: output projection =================
              with (
                  tc.tile_pool(name="oout", bufs=3) as pou,
                  tc.tile_pool(name="ops", bufs=2, space="PSUM") as ops,
              ):
                  for tq in range(8):
                      q0 = 128 * tq
                      for nb2 in range(2):
                          op = ops.tile([128, 512], f32, tag="op")
                          for kc in range(8):
                              nc.tensor.matmul(
                                  op[:],
                                  attn[kc][:, q0 : q0 + 128],
                                  wo_sb[:, 1024 * kc + 512 * nb2 : 1024 * kc + 512 * nb2 + 512],
                                  start=(kc == 0), stop=(kc == 7),
                              )
                          ot = pou.tile([128, 512], f32, tag="ot")
                          nc.vector.tensor_tensor(
                              ot[:], op[:], biasrep[:, 512 * nb2 : 512 * nb2 + 512], ADD
                          )
                          nc.sync.dma_start(
                              out[q0 : q0 + 128, 512 * nb2 : 512 * nb2 + 512], ot[:]
                          )
    nc.finalize()
    return nc


# Q columns: qm[t] hosts heads (t, t+8); per head: [even lanes] + [odd lanes]
_HEAD_ORDER = [t + 8 * p for t in range(8) for p in range(2)]
_PERM_QM = np.concatenate(
    [np.concatenate([64 * h + 2 * np.arange(32), 64 * h + 2 * np.arange(32) + 1])
     for h in _HEAD_ORDER]
)
# K columns: for kv in 0,1: [64kv+2i] + [64kv+2i+1]
_PERM_KM = np.concatenate(
    [np.concatenate([64 * kv + 2 * np.arange(32), 64 * kv + 2 * np.arange(32) + 1])
     for kv in range(KVH)]
)
# 32-row pair-swap permutation (i <-> i^32)
_PERM128 = np.zeros((128, 128), np.float32)
_PERM128[np.arange(128), np.arange(128) ^ 32] = 1.0
# sign pattern for the S' rope tile: -1 on even 32-row groups, +1 on odd
_SGN = np.repeat(np.array([-1.0, 1.0, -1.0, 1.0], np.float32), 32)[:, None]


def make_inputs(x, freqs_cis, w_q, b_q, w_k, b_k, w_v, b_v, w_o, b_o):
    cos = np.asarray(freqs_cis[..., 0], dtype=np.float32)  # (S, 32)
    sin = np.asarray(freqs_cis[..., 1], dtype=np.float32)
    x = np.asarray(x, dtype=np.float32)
    band0 = np.zeros((128, 384), np.float32)
    for k in range(128):
        band0[k, k + 1 : k + 256] = 1.0
    bandm = np.concatenate([band0, band0], axis=1)
    common = dict(
        wqm=np.ascontiguousarray(w_q[:, _PERM_QM]),
        wkm=np.ascontiguousarray(w_k[:, _PERM_KM]),
        wvm=np.ascontiguousarray(w_v),
        wom=np.ascontiguousarray(w_o),
        permm=_PERM128,
        bandm=bandm,
        bqc=np.ascontiguousarray(b_q[_PERM_QM].reshape(8, 128).T).astype(np.float32),
        bkc=np.asarray(b_k[_PERM_KM], np.float32)[:, None],
        bvc=np.asarray(b_v, np.float32)[:, None],
        boc=np.asarray(b_o, np.float32)[None, :],
    )
    maps = []
    for c in range(8):
        b, hf = c // 2, c % 2
        s0 = SL * hf
        pos = s0 - PAD + np.arange(U)
        valid = (pos >= 0) & (pos < S)
        pc = np.clip(pos, 0, S - 1)
        xTc = np.where(valid[None, :], x[b][pc].T, 0.0).astype(np.float32)
        ckc = np.tile(cos[pc].T, (4, 1)).astype(np.float32)
        skc = (np.tile(sin[pc].T, (4, 1)) * _SGN).astype(np.float32)
        qpos = s0 + np.arange(SL)
        cqc = np.tile(cos[qpos].T, (4, 1)).astype(np.float32)
        sqc = (np.tile(sin[qpos].T, (4, 1)) * _SGN).astype(np.float32)
        m = dict(common)
        m.update(xT=xTc, cqt=cqc, sqt=sqc, ckt=ckc, skt=skc)
        maps.append(m)
    return maps


_NC_CACHE = {}


def kernel(x, freqs_cis, w_q, b_q, w_k, b_k, w_v, b_v, w_o, b_o):
    if "nc" not in _NC_CACHE:
        _NC_CACHE["nc"] = build_nc()
    nc = _NC_CACHE["nc"]
    maps = make_inputs(
        np.asarray(x), np.asarray(freqs_cis), np.asarray(w_q), np.asarray(b_q),
        np.asarray(w_k), np.asarray(b_k), np.asarray(w_v), np.asarray(b_v),
        np.asarray(w_o), np.asarray(b_o),
    )
    res = run_bass_kernel_spmd(nc, maps, list(range(8))).results
    full = np.empty((B, S, D), np.float32)
    for c in range(8):
        b, hf = c // 2, c % 2
        full[b, SL * hf : SL * (hf + 1), :] = res[c]["out"]
    return full


# revision 18
# speedup vs baseline: 1.9655x; 1.0350x over previous
"""Banded (sliding-window) GQA attention block on 8 trn2 cores.

Sharding: 8 cores = batch(4) x seq-halves(2). Each core computes 1024
queries for one batch element with a 128-position K/V halo on each side
(window half = 127, padded to 128 so everything is 128-aligned).

Layouts are transposed ([feature, seq]) so the tensor engine contracts
naturally. RoPE even/odd lanes are packed per head into 64 contiguous
partitions ([e0..e31, o0..o31]) so each score block is a single K=64
matmul; the rotation is computed as P1*C + P2*S' where P2 is a 32-row
pair-swap of the projection PSUM obtained with one permutation matmul.

Band masking multiplies the bf16 probabilities with a 0/1 band tile on
the vector engine (2x 16-bit mode) instead of a -inf matmul. Softmax
denominators come from an appended ones-column in V; no max-subtraction
(scores are small enough that raw exp fits in f32).
"""

import sys

sys.path.insert(0, "/opt/trn_rl_repo")

import numpy as np

import concourse.bass as bass
from concourse import bacc
import concourse.mybir as mybir
import concourse.tile as tile
from concourse.bass_utils import run_bass_kernel_spmd
from concourse.masks import make_identity

B, S, D = 4, 2048, 1024
H, KVH, HD = 16, 2, 64
W, HWD = 255, 127
SL = S // 2              # local queries per core
PAD = 128                # left/right key padding (>= half window, 128-aligned)
U = SL + 2 * PAD         # 1280 padded key columns
UQ = U + 256             # 1536: query tensors padded 128 each side
NCH = U // 128           # 10 key chunks

f32 = mybir.dt.float32
f32r = mybir.dt.float32r
bf16 = mybir.dt.bfloat16

Exp = mybir.ActivationFunctionType.Exp
Ident = mybir.ActivationFunctionType.Identity
MULT = mybir.AluOpType.mult
ADD = mybir.AluOpType.add
DIV = mybir.AluOpType.divide


def build_nc():
    nc = bacc.Bacc("TRN2")
    dp = nc.declare_dram_parameter
    xT = dp("xT", [D, U], f32r, isOutput=False)
    wqm = dp("wqm", [D, 1024], f32r, isOutput=False)
    wkm = dp("wkm", [D, 128], f32r, isOutput=False)
    wvm = dp("wvm", [D, 128], f32r, isOutput=False)
    wom = dp("wom", [D, D], f32r, isOutput=False)
    cqt = dp("cqt", [128, SL], f32, isOutput=False)
    sqt = dp("sqt", [128, SL], f32, isOutput=False)
    ckt = dp("ckt", [128, U], f32, isOutput=False)
    skt = dp("skt", [128, U], f32, isOutput=False)
    permm = dp("permm", [128, 128], f32r, isOutput=False)
    bandm = dp("bandm", [128, 768], f32, isOutput=False)
    bqc = dp("bqc", [128, 8], f32, isOutput=False)
    bkc = dp("bkc", [128, 1], f32, isOutput=False)
    bvc = dp("bvc", [128, 1], f32, isOutput=False)
    boc = dp("boc", [1, D], f32r, isOutput=False)
    out = dp("out", [SL, D], f32, isOutput=True)

    NB = [(0, 512), (512, 512), (1024, 256)]  # N-blocks over U
    NBQ = [(0, 512), (512, 512)]              # N-blocks over SL

    with tile.TileContext(nc) as tc:
        with (
            nc.allow_low_precision(reason="f32r tiles are 4-byte; elementwise ops only"),
            tc.tile_pool(name="persist", bufs=1) as pe,
        ):
            # ---- persistent SBUF ----
            ident_f = pe.tile([128, 128], f32, tag="identf")
            make_identity(nc, ident_f)
            ident = pe.tile([128, 128], f32r, tag="ident")
            nc.vector.tensor_copy(ident[:], ident_f[:])
            ones_f = pe.tile([1, 512], f32, tag="onesf")
            nc.vector.memset(ones_f[:], 1.0)
            ones = pe.tile([1, 512], f32r, tag="ones")
            nc.vector.tensor_copy(ones[:], ones_f[:])

            # small parameter tiles (issue DMAs early; tiny transfers)
            bvc_sb = pe.tile([128, 1], f32, tag="bvc")
            bkc_sb = pe.tile([128, 1], f32, tag="bkc")
            bqc_sb = pe.tile([128, 8], f32, tag="bqc")
            bo_sb = pe.tile([1, D], f32r, tag="bo")
            nc.gpsimd.dma_start(bvc_sb[:], bvc[:])
            nc.gpsimd.dma_start(bkc_sb[:], bkc[:])
            nc.gpsimd.dma_start(bqc_sb[:], bqc[:])
            nc.gpsimd.dma_start(bo_sb[:], boc[:])

            qm = [pe.tile([128, UQ], bf16, tag=f"qm{t}", name=f"qm{t}") for t in range(8)]
            ktr = pe.tile([128, U], bf16, tag="ktr")
            vaug = [pe.tile([128, 65 * NCH], bf16, tag=f"vaug{k}", name=f"vaug{k}") for k in range(2)]
            for k in range(2):
                nc.vector.memset(vaug[k][:], 1.0)
            # zero the query padding wings (only cols 256:1280 get written)
            for t in range(8):
                nc.vector.memset(qm[t][:, 0:256], 0.0)
                nc.vector.memset(qm[t][:, UQ - 256 : UQ], 0.0)

            perm_f = pe.tile([128, 128], f32, tag="permf")
            perm_sb = pe.tile([128, 128], bf16, tag="perm")
            band_f = pe.tile([128, 768], f32, tag="bandf")
            band = pe.tile([128, 768], bf16, tag="band")

            # ================= phase A: projections + rope =================
            with (
                tc.tile_pool(name="proj_in", bufs=1) as pin,
                tc.tile_pool(name="ptmp", bufs=1) as ptmp,
            ):
                wv_sb = pin.tile([128, 1024], f32r, tag="wv")
                xts = [pin.tile([128, U], f32r, tag=f"x{i}", name=f"x{i}") for i in range(8)]
                wkm_sb = pin.tile([128, 1024], f32r, tag="wkm")
                wqm_sb = pin.tile([128, 8192], f32r, tag="wqm")
                cq_f = pin.tile([128, SL], f32, tag="cqf")
                sq_f = pin.tile([128, SL], f32, tag="sqf")
                ck_f = pin.tile([128, U], f32, tag="ckf")
                sk_f = pin.tile([128, U], f32, tag="skf")
                cq_sb = pin.tile([128, SL], bf16, tag="cq")
                sq_sb = pin.tile([128, SL], bf16, tag="sq")
                ck_sb = pin.tile([128, U], bf16, tag="ck")
                sk_sb = pin.tile([128, U], bf16, tag="sk")

                # DMA issue order == just-in-time consumption order
                v_kc = lambda kc: slice(128 * kc, 128 * kc + 128)
                nc.gpsimd.dma_start(
                    wv_sb[:].rearrange("p (kc f) -> p kc f", kc=8),
                    wvm[:].rearrange("(kc p) f -> p kc f", kc=8),
                )
                for i in range(4):
                    nc.gpsimd.dma_start(xts[i][:], xT[v_kc(i), :])
                nc.gpsimd.dma_start(
                    wkm_sb[:].rearrange("p (kc f) -> p kc f", kc=8),
                    wkm[:].rearrange("(kc p) f -> p kc f", kc=8),
                )
                for i in range(4, 8):
                    nc.gpsimd.dma_start(xts[i][:], xT[v_kc(i), :])
                nc.gpsimd.dma_start(ck_f[:], ckt[:])
                nc.gpsimd.dma_start(sk_f[:], skt[:])
                nc.gpsimd.dma_start(perm_f[:], permm[:])
                nc.scalar.copy(ck_sb[:], ck_f[:])
                nc.scalar.copy(sk_sb[:], sk_f[:])
                nc.vector.tensor_copy(perm_sb[:], perm_f[:])
                for i in range(8):
                    nc.gpsimd.dma_start(wqm_sb[:, 1024 * i : 1024 * i + 1024], wqm[v_kc(i), :])
                    if i == 1:
                        nc.gpsimd.dma_start(cq_f[:], cqt[:])
                        nc.gpsimd.dma_start(sq_f[:], sqt[:])
                nc.gpsimd.dma_start(band_f[:], bandm[:])
                nc.vector.tensor_copy(band[:], band_f[:])
                nc.scalar.copy(cq_sb[:], cq_f[:])
                nc.scalar.copy(sq_sb[:], sq_f[:])

                # ---- V projection, transposed: Vt[vd, seq] then per-chunk T ----
                with (
                    tc.tile_pool(name="vps", bufs=1, space="PSUM") as vps,
                    tc.tile_pool(name="tps", bufs=2, space="PSUM") as tps,
                ):
                    vt_ps = vps.tile([128, U], f32, tag="vt")
                    for n0, nw in NB:
                        for kc in range(8):
                            nc.tensor.matmul(
                                vt_ps[:, n0 : n0 + nw],
                                wv_sb[:, v_kc(kc)],
                                xts[kc][:, n0 : n0 + nw],
                                start=(kc == 0),
                                stop=(kc == 7),
                            )
                    vt_sb = pin.tile([128, U], f32r, tag="vts")
                    nc.scalar.activation(vt_sb[:], vt_ps[:], Ident, bias=bvc_sb[:, 0:1])
                    for g in range(3):
                        tp = tps.tile([128, 512], f32r, tag="tp")
                        for s in range(8 if g < 2 else 4):
                            st, kvh = (8 * g + s) // 2, (8 * g + s) % 2
                            # identity block at the same partition base as the input
                            isl = slice(64 * kvh, 64 * kvh + 64)
                            nc.tensor.matmul(
                                tp[:, 64 * s : 64 * s + 64],
                                vt_sb[isl, 128 * st : 128 * st + 128],
                                ident[isl, isl],
                                is_transpose=True,
                            )
                            nc.scalar.copy(
                                vaug[kvh][:, 65 * st : 65 * st + 64],
                                tp[:, 64 * s : 64 * s + 64],
                            )

                # ---- K projection (both kv heads, merged e/o lanes) ----
                with tc.tile_pool(name="kps", bufs=1, space="PSUM") as kps:
                    p1k = kps.tile([128, U], f32, tag="p1k")
                    for n0, nw in NB:
                        for kc in range(8):
                            nc.tensor.matmul(
                                p1k[:, n0 : n0 + nw],
                                wkm_sb[:, v_kc(kc)],
                                xts[kc][:, n0 : n0 + nw],
                                start=(kc == 0),
                                stop=(kc == 7),
                            )
                    p1k_sb = pin.tile([128, U], bf16, tag="p1ks")
                    nc.scalar.activation(p1k_sb[:], p1k[:], Ident, bias=bkc_sb[:, 0:1])
                    p2k = kps.tile([128, U], f32, tag="p2k")
                    for n0, nw in NB:
                        nc.tensor.matmul(
                            p2k[:, n0 : n0 + nw], perm_sb[:], p1k_sb[:, n0 : n0 + nw],
                            start=True, stop=True,
                        )
                    p2k_sb = pin.tile([128, U], bf16, tag="p2ks")
                    nc.scalar.copy(p2k_sb[:], p2k[:])
                    tk1 = ptmp.tile([128, U], bf16, tag="tk1")
                    tk2 = ptmp.tile([128, U], bf16, tag="tk2")
                    nc.vector.tensor_tensor(tk1[:], p1k_sb[:], ck_sb[:], MULT)
                    nc.vector.tensor_tensor(tk2[:], p2k_sb[:], sk_sb[:], MULT)
                    nc.vector.tensor_tensor(ktr[:], tk1[:], tk2[:], ADD)

                # ---- Q projection: 8 merged tiles (2 heads each) ----
                with tc.tile_pool(name="qps", bufs=2, space="PSUM") as qps:
                    for t in range(8):
                        p1 = qps.tile([128, SL], f32, tag="p1")
                        for n0, nw in NBQ:
                            for kc in range(8):
                                nc.tensor.matmul(
                                    p1[:, n0 : n0 + nw],
                                    wqm_sb[:, 1024 * kc + 128 * t : 1024 * kc + 128 * t + 128],
                                    xts[kc][:, 128 + n0 : 128 + n0 + nw],
                                    start=(kc == 0),
                                    stop=(kc == 7),
                                )
                        p1_sb = ptmp.tile([128, SL], bf16, tag="p1s")
                        nc.scalar.activation(p1_sb[:], p1[:], Ident, bias=bqc_sb[:, t : t + 1])
                        p2 = qps.tile([128, SL], f32, tag="p2")
                        for n0, nw in NBQ:
                            nc.tensor.matmul(
                                p2[:, n0 : n0 + nw], perm_sb[:], p1_sb[:, n0 : n0 + nw],
                                start=True, stop=True,
                            )
                        p2_sb = ptmp.tile([128, SL], bf16, tag="p2s")
                        nc.scalar.copy(p2_sb[:], p2[:])
                        t1 = ptmp.tile([128, SL], bf16, tag="t1")
                        t2 = ptmp.tile([128, SL], bf16, tag="t2")
                        nc.vector.tensor_tensor(t1[:], p1_sb[:], cq_sb[:], MULT)
                        nc.vector.tensor_tensor(t2[:], p2_sb[:], sq_sb[:], MULT)
                        nc.vector.tensor_tensor(qm[t][:, 256 : 256 + SL], t1[:], t2[:], ADD)

            # ============ phase B: scores -> exp -> mask -> PV -> normalize ============
            with tc.tile_pool(name="pattn", bufs=1) as pattn:
              attn = [pattn.tile([128, SL], f32r, tag=f"attn{t}", name=f"attn{t}") for t in range(8)]
              wo_sb = pattn.tile([128, 8192], f32r, tag="wo")
              for i in range(8):
                  nc.gpsimd.dma_start(
                      wo_sb[:, 1024 * i : 1024 * i + 1024], wom[128 * i : 128 * i + 128, :]
                  )
              biasrep = pattn.tile([128, 1024], f32r, tag="brep")
              with (
                tc.tile_pool(name="spool", bufs=2, space="PSUM") as spool,
                tc.tile_pool(name="ppool", bufs=6) as ppool,
                tc.tile_pool(name="npool", bufs=4) as npool,
                tc.tile_pool(name="ppv", bufs=3, space="PSUM") as ppv,
                tc.tile_pool(name="prb", bufs=1, space="PSUM") as prb,
              ):
                # bias-replica for phase C (built once on PE + act)
                br_ps = prb.tile([128, 512], f32, tag="rb")
                for nb2 in range(2):
                    nc.tensor.matmul(
                        br_ps[:], ones[0:1, 0:128], bo_sb[0:1, 512 * nb2 : 512 * nb2 + 512],
                        start=True, stop=True,
                    )
                    nc.scalar.copy(biasrep[:, 512 * nb2 : 512 * nb2 + 512], br_ps[:])

                deferred = []
                for h in range(H):
                    # qm[t] hosts heads (t, t+8): a head's 64 query lanes sit at
                    # partition base 64*kv, matching its kv rows in ktr.
                    t, kv = h % 8, h // 8
                    r0 = 64 * kv
                    at, ar0 = h // 2, 64 * (h % 2)  # attn feature rows for head h
                    pts = {}
                    pv_ps = [ppv.tile([128, 512], f32, tag="pv", name=f"pv{h}_{m}") for m in range(2)]
                    rb = prb.tile([128, 512], f32, tag="rb")

                    def sc(p):
                        sp = spool.tile([128, 1024], f32, tag="sc")
                        for half in range(2):
                            c = 2 * p + half
                            nc.tensor.matmul(
                                sp[:, 512 * half : 512 * half + 384],
                                ktr[64 * kv : 64 * kv + 64, 128 * c : 128 * c + 128],
                                qm[t][r0 : r0 + 64, 128 * c : 128 * c + 384],
                                start=True, stop=True,
                            )
                        pt = ppool.tile([128, 768], bf16, tag="pt")
                        nc.scalar.activation(
                            pt[:].rearrange("p (b x) -> p b x", b=2),
                            sp[:].rearrange("p (b x) -> p b x", b=2)[:, :, 0:384],
                            Exp,
                        )
                        nc.vector.tensor_tensor(pt[:], pt[:], band[:], MULT)
                        pts[p] = pt

                    def pv(j):
                        m, sl8 = (j - 1) // 4, 128 * ((j - 1) % 4)
                        for c in (j - 1, j, j + 1):
                            nc.tensor.matmul(
                                pv_ps[m][0:65, sl8 : sl8 + 128],
                                vaug[kv][:, 65 * c : 65 * c + 65],
                                pts[c // 2][:, 384 * (c % 2) + 128 * (j - c + 1) :
                                             384 * (c % 2) + 128 * (j - c + 1) + 128],
                                start=(c == j - 1),
                                stop=(c == j + 1),
                            )

                    def den_copy(m):
                        # denominator row (ones-column of V-augmented PV) -> SBUF
                        rd = npool.tile([1, 512], f32r, tag="rd")
                        nc.vector.tensor_copy(rd[0:1, :], pv_ps[m][64:65, 0:512])
                        return rd

                    def rb_div(m, rd, pv_t, a, a0):
                        # broadcast den over 64 partitions, then divide on Pool
                        nc.tensor.matmul(
                            rb[64 * m : 64 * m + 64, :], ones[0:1, 0:64], rd[0:1, :],
                            start=True, stop=True,
                        )
                        nc.gpsimd.tensor_tensor(
                            attn[a][a0 : a0 + 64, 512 * m : 512 * m + 512],
                            pv_t[0:64, 0:512],
                            rb[64 * m : 64 * m + 64, :],
                            DIV,
                        )

                    sc(0)
                    sc(1)
                    for fn in deferred:
                        fn()
                    deferred = []
                    pv(1); pv(2)
                    sc(2)
                    pv(3); pv(4)
                    rd0 = den_copy(0)
                    sc(3)
                    pv(5); pv(6)
                    rb_div(0, rd0, pv_ps[0], at, ar0)
                    sc(4)
                    pv(7); pv(8)
                    rd1 = den_copy(1)
                    deferred.append(
                        lambda m=1, rd=rd1, pv_t=pv_ps[1], a=at, a0=ar0: rb_div(m, rd, pv_t, a, a0)
                    )
                for fn in deferred:
                    fn()

              # ================= phase C: output projection =================
              with (
                  tc.tile_pool(name="oout", bufs=3) as pou,
                  tc.tile_pool(name="ops", bufs=2, space="PSUM") as ops,
              ):
                  for tq in range(8):
                      q0 = 128 * tq
                      for nb2 in range(2):
                          op = ops.tile([128, 512], f32, tag="op")
                          for kc in range(8):
                              nc.tensor.matmul(
                                  op[:],
                                  attn[kc][:, q0 : q0 + 128],
                                  wo_sb[:, 1024 * kc + 512 * nb2 : 1024 * kc + 512 * nb2 + 512],
                                  start=(kc == 0), stop=(kc == 7),
                              )
                          ot = pou.tile([128, 512], f32, tag="ot")
                          nc.vector.tensor_tensor(
                              ot[:], op[:], biasrep[:, 512 * nb2 : 512 * nb2 + 512], ADD
                          )
                          nc.sync.dma_start(
                              out[q0 : q0 + 128, 512 * nb2 : 512 * nb2 + 512], ot[:]
                          )
    nc.finalize()
    return nc


# Q columns: qm[t] hosts heads (t, t+8); per head: [even lanes] + [odd lanes]
_HEAD_ORDER = [t + 8 * p for t in range(8) for p in range(2)]
_PERM_QM = np.concatenate(
    [np.concatenate([64 * h + 2 * np.arange(32), 64 * h + 2 * np.arange(32) + 1])
     for h in _HEAD_ORDER]
)
# K columns: for kv in 0,1: [64kv+2i] + [64kv+2i+1]
_PERM_KM = np.concatenate(
    [np.concatenate([64 * kv + 2 * np.arange(32), 64 * kv + 2 * np.arange(32) + 1])
     for kv in range(KVH)]
)
# 32-row pair-swap permutation (i <-> i^32)
_PERM128 = np.zeros((128, 128), np.float32)
_PERM128[np.arange(128), np.arange(128) ^ 32] = 1.0
# sign pattern for the S' rope tile: -1 on even 32-row groups, +1 on odd
_SGN = np.repeat(np.array([-1.0, 1.0, -1.0, 1.0], np.float32), 32)[:, None]


def make_inputs(x, freqs_cis, w_q, b_q, w_k, b_k, w_v, b_v, w_o, b_o):
    cos = np.asarray(freqs_cis[..., 0], dtype=np.float32)  # (S, 32)
    sin = np.asarray(freqs_cis[..., 1], dtype=np.float32)
    x = np.asarray(x, dtype=np.float32)
    band0 = np.zeros((128, 384), np.float32)
    for k in range(128):
        band0[k, k + 1 : k + 256] = 1.0
    bandm = np.concatenate([band0, band0], axis=1)
    common = dict(
        wqm=np.ascontiguousarray(w_q[:, _PERM_QM]),
        wkm=np.ascontiguousarray(w_k[:, _PERM_KM]),
        wvm=np.ascontiguousarray(w_v),
        wom=np.ascontiguousarray(w_o),
        permm=_PERM128,
        bandm=bandm,
        bqc=np.ascontiguousarray(b_q[_PERM_QM].reshape(8, 128).T).astype(np.float32),
        bkc=np.asarray(b_k[_PERM_KM], np.float32)[:, None],
        bvc=np.asarray(b_v, np.float32)[:, None],
        boc=np.asarray(b_o, np.float32)[None, :],
    )
    maps = []
    for c in range(8):
        b, hf = c // 2, c % 2
        s0 = SL * hf
        pos = s0 - PAD + np.arange(U)
        valid = (pos >= 0) & (pos < S)
        pc = np.clip(pos, 0, S - 1)
        xTc = np.where(valid[None, :], x[b][pc].T, 0.0).astype(np.float32)
        ckc = np.tile(cos[pc].T, (4, 1)).astype(np.float32)
        skc = (np.tile(sin[pc].T, (4, 1)) * _SGN).astype(np.float32)
        qpos = s0 + np.arange(SL)
        cqc = np.tile(cos[qpos].T, (4, 1)).astype(np.float32)
        sqc = (np.tile(sin[qpos].T, (4, 1)) * _SGN).astype(np.float32)
        m = dict(common)
        m.update(xT=xTc, cqt=cqc, sqt=sqc, ckt=ckc, skt=skc)
        maps.append(m)
    return maps


_NC_CACHE = {}


def kernel(x, freqs_cis, w_q, b_q, w_k, b_k, w_v, b_v, w_o, b_o):
    if "nc" not in _NC_CACHE:
        _NC_CACHE["nc"] = build_nc()
    nc = _NC_CACHE["nc"]
    maps = make_inputs(
        np.asarray(x), np.asarray(freqs_cis), np.asarray(w_q), np.asarray(b_q),
        np.asarray(w_k), np.asarray(b_k), np.asarray(w_v), np.asarray(b_v),
        np.asarray(w_o), np.asarray(b_o),
    )
    res = run_bass_kernel_spmd(nc, maps, list(range(8))).results
    full = np.empty((B, S, D), np.float32)
    for c in range(8):
        b, hf = c // 2, c % 2
        full[b, SL * hf : SL * (hf + 1), :] = res[c]["out"]
    return full


# revision 20
# speedup vs baseline: 2.0866x; 1.0616x over previous
"""Banded (sliding-window) GQA attention block on 8 trn2 cores.

Sharding: 8 cores = batch(4) x seq-halves(2). Each core computes 1024
queries for one batch element with a 128-position K/V halo on each side
(window half = 127, padded to 128 so everything is 128-aligned).

Layouts are transposed ([feature, seq]) so the tensor engine contracts
naturally. RoPE even/odd lanes are packed per head into 64 contiguous
partitions ([e0..e31, o0..o31]) so each score block is a single K=64
matmul; the rotation is computed as P1*C + P2*S' where P2 is a 32-row
pair-swap of the projection PSUM obtained with one permutation matmul.

Band masking multiplies the bf16 probabilities with a 0/1 band tile on
the vector engine (2x 16-bit mode) instead of a -inf matmul. Softmax
denominators come from an appended ones-column in V; no max-subtraction
(scores are small enough that raw exp fits in f32).
"""

import sys

sys.path.insert(0, "/opt/trn_rl_repo")

import numpy as np

import concourse.bass as bass
from concourse import bacc
import concourse.mybir as mybir
import concourse.tile as tile
from concourse.bass_utils import run_bass_kernel_spmd
from concourse.masks import make_identity

B, S, D = 4, 2048, 1024
H, KVH, HD = 16, 2, 64
W, HWD = 255, 127
SL = S // 2              # local queries per core
PAD = 128                # left/right key padding (>= half window, 128-aligned)
U = SL + 2 * PAD         # 1280 padded key columns
UQ = U + 256             # 1536: query tensors padded 128 each side
NCH = U // 128           # 10 key chunks

f32 = mybir.dt.float32
f32r = mybir.dt.float32r
bf16 = mybir.dt.bfloat16

Exp = mybir.ActivationFunctionType.Exp
Ident = mybir.ActivationFunctionType.Identity
MULT = mybir.AluOpType.mult
ADD = mybir.AluOpType.add
DIV = mybir.AluOpType.divide


def build_nc():
    nc = bacc.Bacc("TRN2")
    dp = nc.declare_dram_parameter
    xT = dp("xT", [D, U], bf16, isOutput=False)
    wqm = dp("wqm", [128, 8192], bf16, isOutput=False)
    wkm = dp("wkm", [128, 1024], bf16, isOutput=False)
    wvm = dp("wvm", [128, 1024], bf16, isOutput=False)
    wom = dp("wom", [D, D], f32r, isOutput=False)
    cqt = dp("cqt", [128, SL], bf16, isOutput=False)
    sqt = dp("sqt", [128, SL], bf16, isOutput=False)
    ckt = dp("ckt", [128, U], bf16, isOutput=False)
    skt = dp("skt", [128, U], bf16, isOutput=False)
    permm = dp("permm", [128, 128], bf16, isOutput=False)
    bandm = dp("bandm", [128, 768], bf16, isOutput=False)
    bqc = dp("bqc", [128, 8], f32, isOutput=False)
    bkc = dp("bkc", [128, 1], f32, isOutput=False)
    bvc = dp("bvc", [128, 1], f32, isOutput=False)
    boc = dp("boc", [1, D], f32r, isOutput=False)
    out = dp("out", [SL, D], f32, isOutput=True)

    NB = [(0, 512), (512, 512), (1024, 256)]  # N-blocks over U
    NBQ = [(0, 512), (512, 512)]              # N-blocks over SL

    with tile.TileContext(nc) as tc:
        with (
            nc.allow_low_precision(reason="f32r tiles are 4-byte; elementwise ops only"),
            tc.tile_pool(name="persist", bufs=1) as pe,
        ):
            # ---- persistent SBUF ----
            ident_f = pe.tile([128, 128], f32, tag="identf")
            make_identity(nc, ident_f)
            ident = pe.tile([128, 128], f32r, tag="ident")
            nc.vector.tensor_copy(ident[:], ident_f[:])
            ones_f = pe.tile([1, 512], f32, tag="onesf")
            nc.vector.memset(ones_f[:], 1.0)
            ones = pe.tile([1, 512], f32r, tag="ones")
            nc.vector.tensor_copy(ones[:], ones_f[:])

            # small parameter tiles (issue DMAs early; tiny transfers)
            bvc_sb = pe.tile([128, 1], f32, tag="bvc")
            bkc_sb = pe.tile([128, 1], f32, tag="bkc")
            bqc_sb = pe.tile([128, 8], f32, tag="bqc")
            bo_sb = pe.tile([1, D], f32r, tag="bo")
            nc.gpsimd.dma_start(bvc_sb[:], bvc[:])
            nc.gpsimd.dma_start(bkc_sb[:], bkc[:])
            nc.gpsimd.dma_start(bqc_sb[:], bqc[:])
            nc.gpsimd.dma_start(bo_sb[:], boc[:])

            qm = [pe.tile([128, UQ], bf16, tag=f"qm{t}", name=f"qm{t}") for t in range(8)]
            ktr = pe.tile([128, U], bf16, tag="ktr")
            vaug = [pe.tile([128, 65 * NCH], bf16, tag=f"vaug{k}", name=f"vaug{k}") for k in range(2)]
            for k in range(2):
                nc.vector.memset(vaug[k][:], 1.0)
            # zero the query padding wings (only cols 256:1280 get written)
            for t in range(8):
                nc.vector.memset(qm[t][:, 0:256], 0.0)
                nc.vector.memset(qm[t][:, UQ - 256 : UQ], 0.0)

            perm_sb = pe.tile([128, 128], bf16, tag="perm")
            band = pe.tile([128, 768], bf16, tag="band")

            # ================= phase A: projections + rope =================
            with (
                tc.tile_pool(name="proj_in", bufs=1) as pin,
                tc.tile_pool(name="ptmp", bufs=1) as ptmp,
            ):
                wv_sb = pin.tile([128, 1024], bf16, tag="wv")
                xts = [pin.tile([128, U], bf16, tag=f"x{i}", name=f"x{i}") for i in range(8)]
                wkm_sb = pin.tile([128, 1024], bf16, tag="wkm")
                wqm_sb = pin.tile([128, 8192], bf16, tag="wqm")
                cq_sb = pin.tile([128, SL], bf16, tag="cq")
                sq_sb = pin.tile([128, SL], bf16, tag="sq")
                ck_sb = pin.tile([128, U], bf16, tag="ck")
                sk_sb = pin.tile([128, U], bf16, tag="sk")

                # DMA issue order == just-in-time consumption order
                v_kc = lambda kc: slice(128 * kc, 128 * kc + 128)
                nc.gpsimd.dma_start(wv_sb[:], wvm[:])
                for i in range(4):
                    nc.gpsimd.dma_start(xts[i][:], xT[v_kc(i), :])
                nc.gpsimd.dma_start(wkm_sb[:], wkm[:])
                for i in range(4, 8):
                    nc.gpsimd.dma_start(xts[i][:], xT[v_kc(i), :])
                nc.gpsimd.dma_start(ck_sb[:], ckt[:])
                nc.gpsimd.dma_start(sk_sb[:], skt[:])
                nc.gpsimd.dma_start(perm_sb[:], permm[:])
                for i in range(8):
                    nc.gpsimd.dma_start(
                        wqm_sb[:, 1024 * i : 1024 * i + 1024], wqm[:, 1024 * i : 1024 * i + 1024]
                    )
                    if i == 1:
                        nc.gpsimd.dma_start(cq_sb[:], cqt[:])
                        nc.gpsimd.dma_start(sq_sb[:], sqt[:])
                nc.gpsimd.dma_start(band[:], bandm[:])

                # ---- V projection, transposed: Vt[vd, seq] then per-chunk T ----
                with (
                    tc.tile_pool(name="vps", bufs=1, space="PSUM") as vps,
                    tc.tile_pool(name="tps", bufs=2, space="PSUM") as tps,
                ):
                    vt_ps = vps.tile([128, U], f32, tag="vt")
                    for n0, nw in NB:
                        for kc in range(8):
                            nc.tensor.matmul(
                                vt_ps[:, n0 : n0 + nw],
                                wv_sb[:, v_kc(kc)],
                                xts[kc][:, n0 : n0 + nw],
                                start=(kc == 0),
                                stop=(kc == 7),
                            )
                    vt_sb = pin.tile([128, U], f32r, tag="vts")
                    nc.scalar.activation(vt_sb[:], vt_ps[:], Ident, bias=bvc_sb[:, 0:1])
                    for g in range(3):
                        tp = tps.tile([128, 512], f32r, tag="tp")
                        for s in range(8 if g < 2 else 4):
                            st, kvh = (8 * g + s) // 2, (8 * g + s) % 2
                            # identity block at the same partition base as the input
                            isl = slice(64 * kvh, 64 * kvh + 64)
                            nc.tensor.matmul(
                                tp[:, 64 * s : 64 * s + 64],
                                vt_sb[isl, 128 * st : 128 * st + 128],
                                ident[isl, isl],
                                is_transpose=True,
                            )
                            nc.scalar.copy(
                                vaug[kvh][:, 65 * st : 65 * st + 64],
                                tp[:, 64 * s : 64 * s + 64],
                            )

                # ---- K projection (both kv heads, merged e/o lanes) ----
                with tc.tile_pool(name="kps", bufs=1, space="PSUM") as kps:
                    p1k = kps.tile([128, U], f32, tag="p1k")
                    for n0, nw in NB:
                        for kc in range(8):
                            nc.tensor.matmul(
                                p1k[:, n0 : n0 + nw],
                                wkm_sb[:, v_kc(kc)],
                                xts[kc][:, n0 : n0 + nw],
                                start=(kc == 0),
                                stop=(kc == 7),
                            )
                    p1k_sb = pin.tile([128, U], bf16, tag="p1ks")
                    nc.scalar.activation(p1k_sb[:], p1k[:], Ident, bias=bkc_sb[:, 0:1])
                    p2k = kps.tile([128, U], f32, tag="p2k")
                    for n0, nw in NB:
                        nc.tensor.matmul(
                            p2k[:, n0 : n0 + nw], perm_sb[:], p1k_sb[:, n0 : n0 + nw],
                            start=True, stop=True,
                        )
                    p2k_sb = pin.tile([128, U], bf16, tag="p2ks")
                    nc.scalar.copy(p2k_sb[:], p2k[:])
                    tk1 = ptmp.tile([128, U], bf16, tag="tk1")
                    tk2 = ptmp.tile([128, U], bf16, tag="tk2")
                    nc.vector.tensor_tensor(tk1[:], p1k_sb[:], ck_sb[:], MULT)
                    nc.vector.tensor_tensor(tk2[:], p2k_sb[:], sk_sb[:], MULT)
                    nc.vector.tensor_tensor(ktr[:], tk1[:], tk2[:], ADD)

                # ---- Q projection: 8 merged tiles (2 heads each) ----
                with tc.tile_pool(name="qps", bufs=2, space="PSUM") as qps:
                    for t in range(8):
                        p1 = qps.tile([128, SL], f32, tag="p1")
                        for n0, nw in NBQ:
                            for kc in range(8):
                                nc.tensor.matmul(
                                    p1[:, n0 : n0 + nw],
                                    wqm_sb[:, 1024 * t + 128 * kc : 1024 * t + 128 * kc + 128],
                                    xts[kc][:, 128 + n0 : 128 + n0 + nw],
                                    start=(kc == 0),
                                    stop=(kc == 7),
                                )
                        p1_sb = ptmp.tile([128, SL], bf16, tag="p1s")
                        nc.scalar.activation(p1_sb[:], p1[:], Ident, bias=bqc_sb[:, t : t + 1])
                        p2 = qps.tile([128, SL], f32, tag="p2")
                        for n0, nw in NBQ:
                            nc.tensor.matmul(
                                p2[:, n0 : n0 + nw], perm_sb[:], p1_sb[:, n0 : n0 + nw],
                                start=True, stop=True,
                            )
                        p2_sb = ptmp.tile([128, SL], bf16, tag="p2s")
                        nc.scalar.copy(p2_sb[:], p2[:])
                        t1 = ptmp.tile([128, SL], bf16, tag="t1")
                        t2 = ptmp.tile([128, SL], bf16, tag="t2")
                        nc.vector.tensor_tensor(t1[:], p1_sb[:], cq_sb[:], MULT)
                        nc.vector.tensor_tensor(t2[:], p2_sb[:], sq_sb[:], MULT)
                        nc.vector.tensor_tensor(qm[t][:, 256 : 256 + SL], t1[:], t2[:], ADD)

            # ============ phase B: scores -> exp -> mask -> PV -> normalize ============
            with tc.tile_pool(name="pattn", bufs=1) as pattn:
              attn = [pattn.tile([128, SL], f32r, tag=f"attn{t}", name=f"attn{t}") for t in range(8)]
              wo_sb = pattn.tile([128, 8192], f32r, tag="wo")
              for i in range(8):
                  nc.gpsimd.dma_start(
                      wo_sb[:, 1024 * i : 1024 * i + 1024], wom[128 * i : 128 * i + 128, :]
                  )
              biasrep = pattn.tile([128, 1024], f32r, tag="brep")
              with (
                tc.tile_pool(name="spool", bufs=2, space="PSUM") as spool,
                tc.tile_pool(name="ppool", bufs=6) as ppool,
                tc.tile_pool(name="npool", bufs=4) as npool,
                tc.tile_pool(name="ppv", bufs=3, space="PSUM") as ppv,
              ):
                # bias-replica for phase C (built once on PE + act)
                br_ps = ppv.tile([128, 512], f32, tag="pv")
                for nb2 in range(2):
                    nc.tensor.matmul(
                        br_ps[:], ones[0:1, 0:128], bo_sb[0:1, 512 * nb2 : 512 * nb2 + 512],
                        start=True, stop=True,
                    )
                    nc.scalar.copy(biasrep[:, 512 * nb2 : 512 * nb2 + 512], br_ps[:])

                deferred = []
                for h in range(H):
                    # qm[t] hosts heads (t, t+8): a head's 64 query lanes sit at
                    # partition base 64*kv, matching its kv rows in ktr.
                    t, kv = h % 8, h // 8
                    r0 = 64 * kv
                    at, ar0 = h // 2, 64 * (h % 2)  # attn feature rows for head h
                    pts = {}
                    pv_ps = [ppv.tile([128, 512], f32, tag="pv", name=f"pv{h}_{m}") for m in range(2)]

                    def sc(p):
                        sp = spool.tile([128, 1024], f32, tag="sc")
                        for half in range(2):
                            c = 2 * p + half
                            nc.tensor.matmul(
                                sp[:, 512 * half : 512 * half + 384],
                                ktr[64 * kv : 64 * kv + 64, 128 * c : 128 * c + 128],
                                qm[t][r0 : r0 + 64, 128 * c : 128 * c + 384],
                                start=True, stop=True,
                            )
                        pt = ppool.tile([128, 768], bf16, tag="pt")
                        nc.scalar.activation(
                            pt[:].rearrange("p (b x) -> p b x", b=2),
                            sp[:].rearrange("p (b x) -> p b x", b=2)[:, :, 0:384],
                            Exp,
                        )
                        nc.vector.tensor_tensor(pt[:], pt[:], band[:], MULT)
                        pts[p] = pt

                    def pv(j):
                        m, sl8 = (j - 1) // 4, 128 * ((j - 1) % 4)
                        for c in (j - 1, j, j + 1):
                            nc.tensor.matmul(
                                pv_ps[m][0:65, sl8 : sl8 + 128],
                                vaug[kv][:, 65 * c : 65 * c + 65],
                                pts[c // 2][:, 384 * (c % 2) + 128 * (j - c + 1) :
                                             384 * (c % 2) + 128 * (j - c + 1) + 128],
                                start=(c == j - 1),
                                stop=(c == j + 1),
                            )

                    def den_copy(m):
                        # denominator row (ones-column of V-augmented PV) -> SBUF
                        rd = npool.tile([1, 512], f32r, tag="rd")
                        nc.vector.tensor_copy(rd[0:1, :], pv_ps[m][64:65, 0:512])
                        return rd

                    def rb_div(m, rd, pv_t, a, a0):
                        # broadcast den into the unused partitions 64:128 of the
                        # pv psum tile (den row 64 was already copied out), then
                        # divide on the Pool engine
                        nc.tensor.matmul(
                            pv_t[64:128, 0:512], ones[0:1, 0:64], rd[0:1, :],
                            start=True, stop=True,
                        )
                        nc.gpsimd.tensor_tensor(
                            attn[a][a0 : a0 + 64, 512 * m : 512 * m + 512],
                            pv_t[0:64, 0:512],
                            pv_t[64:128, 0:512],
                            DIV,
                        )

                    sc(0)
                    sc(1)
                    for fn in deferred:
                        fn()
                    deferred = []
                    pv(1); pv(2)
                    sc(2)
                    pv(3); pv(4)
                    rd0 = den_copy(0)
                    sc(3)
                    pv(5); pv(6)
                    rb_div(0, rd0, pv_ps[0], at, ar0)
                    sc(4)
                    pv(7); pv(8)
                    rd1 = den_copy(1)
                    deferred.append(
                        lambda m=1, rd=rd1, pv_t=pv_ps[1], a=at, a0=ar0: rb_div(m, rd, pv_t, a, a0)
                    )
                for fn in deferred:
                    fn()

              # ================= phase C: output projection =================
              with (
                  tc.tile_pool(name="oout", bufs=3) as pou,
                  tc.tile_pool(name="ops", bufs=2, space="PSUM") as ops,
              ):
                  for tq in range(8):
                      q0 = 128 * tq
                      for nb2 in range(2):
                          op = ops.tile([128, 512], f32, tag="op")
                          for kc in range(8):
                              nc.tensor.matmul(
                                  op[:],
                                  attn[kc][:, q0 : q0 + 128],
                                  wo_sb[:, 1024 * kc + 512 * nb2 : 1024 * kc + 512 * nb2 + 512],
                                  start=(kc == 0), stop=(kc == 7),
                              )
                          ot = pou.tile([128, 512], f32, tag="ot")
                          nc.vector.tensor_tensor(
                              ot[:], op[:], biasrep[:, 512 * nb2 : 512 * nb2 + 512], ADD
                          )
                          nc.sync.dma_start(
                              out[q0 : q0 + 128, 512 * nb2 : 512 * nb2 + 512], ot[:]
                          )
    nc.finalize()
    return nc


# Q columns: qm[t] hosts heads (t, t+8); per head: [even lanes] + [odd lanes]
_HEAD_ORDER = [t + 8 * p for t in range(8) for p in range(2)]
_PERM_QM = np.concatenate(
    [np.concatenate([64 * h + 2 * np.arange(32), 64 * h + 2 * np.arange(32) + 1])
     for h in _HEAD_ORDER]
)
# K columns: for kv in 0,1: [64kv+2i] + [64kv+2i+1]
_PERM_KM = np.concatenate(
    [np.concatenate([64 * kv + 2 * np.arange(32), 64 * kv + 2 * np.arange(32) + 1])
     for kv in range(KVH)]
)
# 32-row pair-swap permutation (i <-> i^32)
_PERM128 = np.zeros((128, 128), np.float32)
_PERM128[np.arange(128), np.arange(128) ^ 32] = 1.0
# sign pattern for the S' rope tile: -1 on even 32-row groups, +1 on odd
_SGN = np.repeat(np.array([-1.0, 1.0, -1.0, 1.0], np.float32), 32)[:, None]


def _chunk_major(w):
    # [D, F] -> [128, 8*F]: column block kc holds rows 128kc..128kc+127
    F = w.shape[1]
    return np.ascontiguousarray(w.reshape(8, 128, F).transpose(1, 0, 2).reshape(128, 8 * F))


def make_inputs(x, freqs_cis, w_q, b_q, w_k, b_k, w_v, b_v, w_o, b_o):
    import ml_dtypes

    BF = ml_dtypes.bfloat16
    cos = np.asarray(freqs_cis[..., 0], dtype=np.float32)  # (S, 32)
    sin = np.asarray(freqs_cis[..., 1], dtype=np.float32)
    x = np.asarray(x, dtype=np.float32)
    band0 = np.zeros((128, 384), np.float32)
    for k in range(128):
        band0[k, k + 1 : k + 256] = 1.0
    bandm = np.concatenate([band0, band0], axis=1)
    # wq: [128, 8192] tile-major: cols 1024t+128kc+j = w[128kc+p, 128t+j]
    wq_p = w_q[:, _PERM_QM]
    wqm = wq_p.reshape(8, 128, 8, 128).transpose(1, 2, 0, 3).reshape(128, 8192)
    common = dict(
        wqm=np.ascontiguousarray(wqm).astype(BF),
        wkm=_chunk_major(w_k[:, _PERM_KM]).astype(BF),
        wvm=_chunk_major(np.asarray(w_v)).astype(BF),
        wom=np.ascontiguousarray(w_o).astype(np.float32),
        permm=_PERM128.astype(BF),
        bandm=bandm.astype(BF),
        bqc=np.ascontiguousarray(b_q[_PERM_QM].reshape(8, 128).T).astype(np.float32),
        bkc=np.asarray(b_k[_PERM_KM], np.float32)[:, None],
        bvc=np.asarray(b_v, np.float32)[:, None],
        boc=np.asarray(b_o, np.float32)[None, :],
    )
    maps = []
    for c in range(8):
        b, hf = c // 2, c % 2
        s0 = SL * hf
        pos = s0 - PAD + np.arange(U)
        valid = (pos >= 0) & (pos < S)
        pc = np.clip(pos, 0, S - 1)
        xTc = np.where(valid[None, :], x[b][pc].T, 0.0).astype(BF)
        ckc = np.tile(cos[pc].T, (4, 1)).astype(BF)
        skc = (np.tile(sin[pc].T, (4, 1)) * _SGN).astype(BF)
        qpos = s0 + np.arange(SL)
        cqc = np.tile(cos[qpos].T, (4, 1)).astype(BF)
        sqc = (np.tile(sin[qpos].T, (4, 1)) * _SGN).astype(BF)
        m = dict(common)
        m.update(xT=xTc, cqt=cqc, sqt=sqc, ckt=ckc, skt=skc)
        maps.append(m)
    return maps


_NC_CACHE = {}


def kernel(x, freqs_cis, w_q, b_q, w_k, b_k, w_v, b_v, w_o, b_o):
    if "nc" not in _NC_CACHE:
        _NC_CACHE["nc"] = build_nc()
    nc = _NC_CACHE["nc"]
    maps = make_inputs(
        np.asarray(x), np.asarray(freqs_cis), np.asarray(w_q), np.asarray(b_q),
        np.asarray(w_k), np.asarray(b_k), np.asarray(w_v), np.asarray(b_v),
        np.asarray(w_o), np.asarray(b_o),
    )
    res = run_bass_kernel_spmd(nc, maps, list(range(8))).results
    full = np.empty((B, S, D), np.float32)
    for c in range(8):
        b, hf = c // 2, c % 2
        full[b, SL * hf : SL * (hf + 1), :] = res[c]["out"]
    return full
